# revision 38
# baseline (speedup 1.0000x reference)
"""Trainium2 Bass kernel for nn_BiGNN_53772990546511.

Strategy (see baseline docstring for the math derivation; this version
restructures for engine overlap):
  - relu(elu(x)) == relu(x); days 2..4 degenerate to row algebra; day-1
    attention is a 1024x1024 user x loc problem with multiplicity mask M.
  - E = f1 (+) f2 outer sum is built by ONE rank-2 matmul per [128,512]
    tile (lhsT=[ones;f2], rhs=[f1;ones]); leaky on GpSimd via
    scalar_tensor_tensor max(E, 0.2E); single Exp on Scalar (bias -10);
    M-mask multiply on Vector.  3-engine pipeline for PT production.
  - Outputs slimmed: device writes day0/day1 user halves (fp16) + 7 rows
    (v1, r1/vn for days 2..4); host expands loc broadcasts and the
    two-valued day>=2 user rows.  Day-0 loc rows are x_loc verbatim.
  - All-fp16 matmuls (fp32 is 4 cyc/row on PE).
  - Input DMAs spread across sync/scalar/vector queues for overlap.

Sharding: 8 cores = 4 batch pairs; both cores of a pair compute the full
per-batch recurrence, each writes half of the user rows (odd cores get
user-axis-rotated index tensors, SPMD-uniform program).
"""
import numpy as np

N_USER = 1024
N_LOC = 1024
DM = 256
HD = 256
B = 4
D = 5
E = 4096
ALPHA = 0.2
EXPBIAS = -10.0
P = 128
NCORES = 8

_CACHE = {}


# --------------------------------------------------------------------------
# Workarounds for this walrus build's 1-sync-wait-per-instruction limit.
# --------------------------------------------------------------------------
def _apply_tile_patch():
    import concourse.tile as tile
    from concourse.tile_sem_assignment import tick_to_sem

    if not getattr(tile.TileContext, "_drain_patched", False):
        def _patched(self, tick_clock, wait_clock):
            nc = self.nc
            gc = tick_clock.global_clock
            for proc, sem in self.sems.allocated().items():
                t = gc[proc]
                if t and t > 0:
                    nc.sync.nop().wait_op(sem, tick_to_sem(t, proc), "sem-ge")
            nc.sync.drain()
            nc.all_engine_barrier()
            popped = nc._tile_sem_poison_stack.pop()
            assert popped is self._sem_poison
            nc.clear_and_free_semaphores(list(self.sems.allocated().values()))
            nc.all_engine_barrier()

        tile.TileContext._drain_and_barrier = _patched
        tile.TileContext._drain_patched = True

    import json as _json
    import concourse.bass_utils as _bu
    import concourse.bass2jax as _b2j

    if not getattr(_bu, "_wait_split_patched", False):
        _orig_compile = _bu.compile_bir_kernel

        def _split_waits(bir_json):
            j = _json.loads(bir_json)
            nid = [0]
            for fn in j.get("functions", []):
                for bb in fn.get("blocks", []):
                    out = []
                    for inst in bb.get("instructions", []):
                        si = inst.get("sync_info") or {}
                        ow = si.get("on_wait") or []
                        if len(ow) > 1:
                            for w in ow[:-1]:
                                nid[0] += 1
                                out.append({
                                    "debug": inst.get("debug", 0),
                                    "engine": inst.get("engine", "SP"),
                                    "ins": [],
                                    "name": f"WSPL-{nid[0]}",
                                    "opcode": "NoOp",
                                    "outs": [],
                                    "sync_info": {"on_update": [],
                                                  "on_wait": [w]},
                                })
                            si["on_wait"] = [ow[-1]]
                        out.append(inst)
                    bb["instructions"] = out
            return _json.dumps(j).encode()

        def _patched_compile(bir_json, tmpdir, neff_name="file.neff"):
            return _orig_compile(_split_waits(bir_json), tmpdir,
                                 neff_name=neff_name)

        _bu.compile_bir_kernel = _patched_compile
        _b2j.compile_bir_kernel = _patched_compile
        _bu._wait_split_patched = True


def _build_nc():
    import concourse.bass as bass
    import concourse.tile as tile
    from concourse import mybir

    _apply_tile_patch()
    f32 = mybir.dt.float32
    f16 = mybir.dt.float16
    f8 = mybir.dt.float8e4
    AF = mybir.ActivationFunctionType
    OP = mybir.AluOpType

    nc = bass.Bass()

    # ---------------- DRAM tensors ----------------
    d_AT = nc.dram_tensor("AT8", [N_LOC, N_USER], f8, kind="ExternalInput")
    d_MT = nc.dram_tensor("MT8", [N_LOC, N_USER], f8, kind="ExternalInput")
    d_cntc = nc.dram_tensor("cntc", [P, 8], f32, kind="ExternalInput")
    d_cnt2 = nc.dram_tensor("cnt2", [2, N_USER], f16, kind="ExternalInput")
    d_xloc8 = nc.dram_tensor("xloc8", [N_LOC, DM], f8, kind="ExternalInput")
    d_xlocT16 = nc.dram_tensor("xlocT16", [DM, N_LOC], f16, kind="ExternalInput")
    d_W16 = nc.dram_tensor("W16", [DM, HD], f16, kind="ExternalInput")
    d_WT16 = nc.dram_tensor("WT16", [HD, DM], f16, kind="ExternalInput")
    d_acol16 = nc.dram_tensor("acol16", [P, 4], f16, kind="ExternalInput")
    d_nothas16 = nc.dram_tensor("nothas16", [1, N_USER], f16,
                                kind="ExternalInput")
    d_uw316 = nc.dram_tensor("uw316", [N_USER, 3], f16, kind="ExternalInput")
    d_hcol3 = nc.dram_tensor("hcol3", [3, 1], f32, kind="ExternalInput")
    d_cpair16 = nc.dram_tensor("cpair16", [2, 1], f16, kind="ExternalInput")
    d_ner16 = nc.dram_tensor("ner16", [1, 6], f16, kind="ExternalInput")
    d_ou = nc.dram_tensor("out_u", [2, 512, HD], f16, kind="ExternalOutput")
    d_rows = nc.dram_tensor("out_rows", [1, 8 * HD], f16,
                            kind="ExternalOutput")

    with tile.TileContext(nc) as tc:
        import contextlib
        with contextlib.ExitStack() as ctx:
            persist = ctx.enter_context(tc.tile_pool(name="persist", bufs=1))
            work = ctx.enter_context(tc.tile_pool(name="work", bufs=1))
            psE = ctx.enter_context(tc.tile_pool(name="psE", bufs=1,
                                                 space="PSUM"))
            psB = ctx.enter_context(tc.tile_pool(name="psB", bufs=1,
                                                 space="PSUM"))
            psC = ctx.enter_context(tc.tile_pool(name="psC", bufs=1,
                                                 space="PSUM"))

            def big_load(eng, dst, dram, t):
                src = dram.rearrange("(t p) u -> p t u", p=P)
                eng.dma_start(
                    out=dst[:].rearrange("p (t u) -> p t u", t=t), in_=src)

            # ------------- load inputs --------------------------------
            # sync queue: small early tensors + first half of A + W
            cntc = persist.tile([P, 8], f32, name="cntc")
            nc.sync.dma_start(out=cntc[:], in_=d_cntc[:])
            cnt2 = persist.tile([2, N_USER], f16, name="cnt2")
            nc.sync.dma_start(out=cnt2[:], in_=d_cnt2[:])
            xlocT16 = persist.tile([P, 2 * N_LOC], f16, name="xlocT16")
            big_load(nc.sync, xlocT16, d_xlocT16[:], 2)
            W16 = persist.tile([P, 2 * HD], f16, name="W16")
            big_load(nc.sync, W16, d_W16[:], 2)
            AT = persist.tile([P, 8 * N_USER], f8, name="AT")
            for c in range(2):
                nc.sync.dma_start(
                    out=AT[:, 2 * c * N_USER:(2 * c + 2) * N_USER].rearrange(
                        "p (t u) -> p t u", t=2),
                    in_=d_AT[2 * c * P:(2 * c + 2) * P, :].rearrange(
                        "(t p) u -> p t u", p=P))
            nothas16 = persist.tile([1, N_USER], f16, name="nothas16")
            nc.sync.dma_start(out=nothas16[:], in_=d_nothas16[:])
            uw316 = persist.tile([P, 8 * 3], f16, name="uw316")
            big_load(nc.sync, uw316, d_uw316[:], 8)
            hcol3 = persist.tile([3, 1], f32, name="hcol3")
            nc.sync.dma_start(out=hcol3[:], in_=d_hcol3[:])
            cpair16 = persist.tile([2, 1], f16, name="cpair16")
            nc.sync.dma_start(out=cpair16[:], in_=d_cpair16[:])
            ner16 = persist.tile([1, 6], f16, name="ner16")
            nc.sync.dma_start(out=ner16[:], in_=d_ner16[:])

            # scalar queue: issues land in scalar's idle pre-exp window
            WT16 = persist.tile([P, 2 * DM], f16, name="WT16")
            big_load(nc.scalar, WT16, d_WT16[:], 2)
            acol16 = persist.tile([P, 4], f16, name="acol16")
            nc.scalar.dma_start(out=acol16[:], in_=d_acol16[:])
            xloc8 = persist.tile([P, 8 * DM], f8, name="xloc8")
            big_load(nc.scalar, xloc8, d_xloc8[:], 8)
            for c in range(2, 4):
                nc.scalar.dma_start(
                    out=AT[:, 2 * c * N_USER:(2 * c + 2) * N_USER].rearrange(
                        "p (t u) -> p t u", t=2),
                    in_=d_AT[2 * c * P:(2 * c + 2) * P, :].rearrange(
                        "(t p) u -> p t u", p=P))
            MT = persist.tile([P, 8 * N_USER], f8, name="MT")
            for half in range(2):
                nc.scalar.dma_start(
                    out=MT[:, half * 4 * N_USER:(half + 1) * 4 * N_USER],
                    in_=d_MT[half * 4 * P:(half + 1) * 4 * P, :].rearrange(
                        "(t p) u -> p t u", p=P))

            # constants
            one11_16 = persist.tile([1, 1], f16, name="one11_16")
            nc.vector.memset(one11_16[:], 1.0)
            ones32c = persist.tile([P, 1], f16, name="ones32c")
            nc.vector.memset(ones32c[:], 1.0)
            l3 = persist.tile([P, 3], f8, name="l3")
            nc.vector.memset(l3[:, 0:2], 0.0)
            nc.vector.memset(l3[:, 2:3], 1.0)
            ebias = persist.tile([P, 1], f32, name="ebias")
            nc.vector.memset(ebias[:], EXPBIAS)

            # act-table preload (Exp) + PE p-state warm-up during DMA wait
            warmup = persist.tile([1, 16], f16, name="warmup")
            nc.vector.memset(warmup[:], 0.0)
            nc.scalar.activation(warmup[:], warmup[:], AF.Exp)
            for _ in range(24):
                wps = psC.tile([3, 1], f32, name="wps", tag="ps1", bufs=2)
                nc.tensor.matmul(wps[:], l3[:], l3[:, 0:1],
                                 start=True, stop=True)

            def W16k(kt):
                return W16[:, kt * HD:(kt + 1) * HD]

            def WT16k(kt):
                return WT16[:, kt * DM:(kt + 1) * DM]

            def xT16(kt, sl):
                return xlocT16[:, kt * N_LOC:(kt + 1) * N_LOC][:, sl]

            DR = mybir.MatmulPerfMode.DoubleRow

            def xl8(lt):
                return xloc8[:, lt * DM:(lt + 1) * DM]

            AT3 = AT[:].rearrange("p (lt u) -> p lt u", lt=8)
            xl3 = xloc8[:].rearrange("p (lt dm) -> p lt dm", lt=8)

            def Ah(lt, sl):
                return AT[:, lt * N_USER:(lt + 1) * N_USER][:, sl]

            def Mk(lt, sl):
                return MT[:, lt * N_USER:(lt + 1) * N_USER][:, sl]

            # ---------------- phase 1: small matmuls ----------------
            # wa1/wa2 = W^T a halves, as fp16 columns
            wa1_16 = [persist.tile([P, 1], f16, name=f"wa1h{i}")
                      for i in range(2)]
            wa2_16 = [persist.tile([P, 1], f16, name=f"wa2h{i}")
                      for i in range(2)]
            for dst, ai in ((wa1_16, 0), (wa2_16, 2)):
                for mt in range(2):
                    ps = psC.tile([P, 1], f32, name="ps1", tag="ps1", bufs=2)
                    for kt in range(2):
                        nc.tensor.matmul(ps[:],
                                         WT16k(kt)[:, mt * P:(mt + 1) * P],
                                         acol16[:, ai + kt:ai + kt + 1],
                                         start=(kt == 0), stop=(kt == 1))
                    nc.vector.tensor_copy(dst[mt][:], ps[:])

            # helper tiles for 2-row [val; ones] PSUM construction
            # (engine writes may only start at partition 0, so both rows of
            # the f1e/f2e operands are built by matmuls writing [2,*] PSUM)
            onesrow16 = persist.tile([1, 512], f16, name="onesrow16")
            nc.vector.memset(onesrow16[:], 1.0)
            z01 = persist.tile([1, 2], f16, name="z01")
            nc.vector.memset(z01[:, 0:1], 0.0)
            nc.vector.memset(z01[:, 1:2], 1.0)
            z10 = persist.tile([1, 2], f16, name="z10")
            nc.vector.memset(z10[:, 0:1], 1.0)
            nc.vector.memset(z10[:, 1:2], 0.0)

            # wa2 column pairs [0 | wa2]
            wa2pair = persist.tile([P, 4], f16, name="wa2pair")
            nc.vector.memset(wa2pair[:], 0.0)
            for kt in range(2):
                nc.vector.tensor_copy(wa2pair[:, 2 * kt + 1:2 * kt + 2],
                                      wa2_16[kt][:])

            for _ in range(10):
                wps = psC.tile([3, 1], f32, name="wps", tag="ps1", bufs=2)
                nc.tensor.matmul(wps[:], l3[:], l3[:, 0:1],
                                 start=True, stop=True)

            # xw1 column pairs [xw1 | 0] (per loc tile): f1 contributions.
            # All 16 matmuls write columns of one PSUM tile; one strided copy
            # drains them into the [xw1 | 0] pair layout.
            xw1all = persist.tile([P, 16], f16, name="xw1all")
            nc.vector.memset(xw1all[:], 0.0)
            psX = psC.tile([P, 8], f32, name="psX", tag="ps1", bufs=2)
            for lt in range(8):
                sl = slice(lt * P, (lt + 1) * P)
                for kt in range(2):
                    nc.tensor.matmul(psX[:, lt:lt + 1], xT16(kt, sl),
                                     wa1_16[kt][:],
                                     start=(kt == 0), stop=(kt == 1))
            nc.vector.tensor_copy(
                xw1all[:].rearrange("p (l two) -> p l two", two=2)[:, :, 0:1],
                psX[:].rearrange("p (l one) -> p l one", one=1))

            # f2e = [ones; f2] (lhsT of the E matmuls)
            f2e = persist.tile([2, N_LOC], f16, name="f2e")
            for ch in range(2):
                csl = slice(ch * 512, (ch + 1) * 512)
                ps = psE.tile([2, 512], f32, name="pe2", tag="pe", bufs=2)
                for kt in range(2):
                    nc.tensor.matmul(ps[:], wa2pair[:, 2 * kt:2 * kt + 2],
                                     xT16(kt, csl), start=(kt == 0),
                                     stop=False)
                nc.tensor.matmul(ps[:], z10[:], onesrow16[:],
                                 start=False, stop=True)
                nc.vector.tensor_copy(f2e[:, csl], ps[:])

            # whext = [Wh | 1] per loc tile (tile alloc here; matmuls are
            # emitted inside the pipe loop to fill PE slack)
            whext = persist.tile([P, 8 * (HD + 1)], f16, name="whext")

            def emit_whext(lt):
                sl = slice(lt * P, (lt + 1) * P)
                base = lt * (HD + 1)
                ps = psB.tile([P, HD + 1], f32, name="pb", tag="pb", bufs=2)
                for kt in range(2):
                    nc.tensor.matmul(ps[:, 0:HD], xT16(kt, sl), W16k(kt),
                                     start=(kt == 0), stop=(kt == 1))
                nc.scalar.copy(whext[:, base:base + HD], ps[:, 0:HD])
                nc.vector.memset(whext[:, base + HD:base + HD + 1], 1.0)

            def whx(lt):
                return whext[:, lt * (HD + 1):(lt + 1) * (HD + 1)]

            # ---------------- phase 4: f1 row -> f1e = [f1; ones] --------
            # the two 512-user chunks accumulate in parallel so both finish
            # as soon as the last A chunk lands
            f1e = persist.tile([2, N_USER], f16, name="f1e")
            pf = [psE.tile([2, 512], f32, name="pe2", tag="pe", bufs=2)
                  for _ in range(2)]
            for lt in range(8):
                for ch in range(2):
                    csl = slice(ch * 512, (ch + 1) * 512)
                    nc.tensor.matmul(pf[ch][:], xw1all[:, 2 * lt:2 * lt + 2],
                                     Ah(lt, csl), start=(lt == 0),
                                     stop=False)
            for ch in range(2):
                csl = slice(ch * 512, (ch + 1) * 512)
                nc.tensor.matmul(pf[ch][:], z01[:], onesrow16[:],
                                 start=False, stop=True)
                nc.vector.scalar_tensor_tensor(
                    out=f1e[:, csl], in0=pf[ch][:], scalar=1.0,
                    in1=cnt2[:, csl], op0=OP.mult, op1=OP.mult)

            # ---- phases 2+5 interleaved: x_user groups + PT production ----
            # PT = M * exp(leaky(E) - 10); E by rank-2 matmul, leaky on
            # GpSimd, exp on Scalar, mask-mult on Vector.  One phase-2
            # accumulation group (8 matmuls) is issued between E pairs so
            # the PE never stalls on the PT pipeline.
            PT = persist.tile([P, 8 * N_USER], f16, name="PT")
            xu16 = persist.tile([P, 8 * DM], f16, name="xu16")
            for i in range(8):
                # E block for loc tile i (first: it feeds the PT pipeline)
                base = i * N_USER
                lsl = slice(i * P, (i + 1) * P)
                osl = slice(base, base + N_USER)
                pe = psE.tile([P, N_USER], f32, name="pe", tag="pe", bufs=2)
                for ch in range(2):
                    csl = slice(ch * 512, (ch + 1) * 512)
                    nc.tensor.matmul(pe[:, csl], f2e[:, lsl], f1e[:, csl],
                                     start=True, stop=True)
                x1 = work.tile([P, N_USER], f16, name="x1", tag="x1", bufs=2)
                x2 = work.tile([P, N_USER], f16, name="x2", tag="x2", bufs=2)
                nc.scalar.activation(x1[:], pe[:], AF.Exp, bias=ebias[:])
                nc.scalar.activation(x2[:], pe[:], AF.Exp, bias=ebias[:],
                                     scale=ALPHA)
                nc.vector.tensor_tensor(out=PT[:, osl], in0=x1[:],
                                        in1=x2[:], op=OP.max)
                meng = nc.gpsimd if i >= 5 else (nc.vector if i % 2 == 1
                                                  else nc.gpsimd)
                meng.tensor_tensor(out=PT[:, osl], in0=PT[:, osl],
                                   in1=Mk(i, slice(0, N_USER)), op=OP.mult)
                # phase-2 group for user tile i: 4 fp8 DoubleRow matmuls
                usl = slice(i * P, (i + 1) * P)
                ps = psB.tile([P, DM], f32, name="pb", tag="pb", bufs=2)
                for t in range(4):
                    nc.tensor.matmul(ps[:], AT3[:, 2 * t:2 * t + 2, usl],
                                     xl3[:, 2 * t:2 * t + 2, :],
                                     start=(t == 0), stop=(t == 3),
                                     perf_mode=DR)
                if i % 2 == 0:
                    nc.vector.tensor_scalar(
                        out=xu16[:, i * DM:(i + 1) * DM], in0=ps[:],
                        scalar1=cntc[:, i:i + 1], scalar2=None, op0=OP.mult)
                else:
                    nc.scalar.activation(xu16[:, i * DM:(i + 1) * DM], ps[:],
                                         AF.Copy, scale=cntc[:, i:i + 1])
                emit_whext(i)

            # ---------------- phase 3: means / day-0 ----------------
            # stack3 rows = [mwe; sxu; sxl]
            ps3 = psB.tile([3, DM], f32, name="ps3", tag="pb", bufs=2)
            for lt in range(8):
                nc.tensor.matmul(ps3[:], l3[:], xl8(lt),
                                 start=(lt == 0), stop=False)
            for ut in range(8):
                nc.tensor.matmul(ps3[:], uw316[:, ut * 3:(ut + 1) * 3],
                                 xu16[:, ut * DM:(ut + 1) * DM],
                                 start=False, stop=(ut == 7))
            stack3 = persist.tile([3, DM], f32, name="stack3")
            nc.vector.tensor_copy(stack3[:], ps3[:])
            mwe16 = persist.tile([1, DM], f16, name="mwe16")
            nc.vector.tensor_copy(mwe16[:], stack3[0:1, :])
            mcol16 = [persist.tile([P, 1], f16, name=f"mcol16_{i}")
                      for i in range(2)]
            for mt in range(2):
                ps = psC.tile([P, 1], f32, name="ps1", tag="ps1", bufs=2)
                nc.tensor.matmul(ps[:], stack3[:, mt * P:(mt + 1) * P],
                                 hcol3[:], start=True, stop=True)
                nc.vector.tensor_copy(mcol16[mt][:], ps[:])
            # mw0 row (mean of Wh0 over all 3072 rows) + ext 1
            mw0e16 = persist.tile([1, HD + 1], f16, name="mw0e16")
            nc.vector.memset(mw0e16[:, HD:HD + 1], 1.0)
            psm = psB.tile([1, 512], f32, name="psr", tag="pb", bufs=2)
            for kt in range(2):
                nc.tensor.matmul(psm[:, 0:HD], mcol16[kt][:], W16k(kt),
                                 start=(kt == 0), stop=(kt == 1))
            nc.vector.tensor_copy(mw0e16[:, 0:HD], psm[:, 0:HD])

            # rows output tile; row 0 = v1 = relu(mw0); row 7 unused
            rows_sb = persist.tile([1, 8 * HD], f16, name="rows_sb")
            nc.vector.memset(rows_sb[:, 7 * HD:], 0.0)
            nc.scalar.activation(rows_sb[:, 0:HD], psm[:, 0:HD], AF.Relu)

            def row_sl(i):
                return rows_sb[0:1, i * HD:(i + 1) * HD]

            # v1 as fp16 columns (for the day-2 recurrence)
            def trans_row_to_col(row16, dst2):
                for mt in range(2):
                    ps = psC.tile([P, 1], f32, name="ps1", tag="ps1", bufs=2)
                    nc.tensor.matmul(ps[:], row16[0:1, mt * P:(mt + 1) * P],
                                     one11_16[:], start=True, stop=True)
                    nc.vector.tensor_copy(dst2[mt][:], ps[:])

            v1c16 = [persist.tile([P, 1], f16, name=f"v1c16_{i}")
                     for i in range(2)]
            trans_row_to_col(row_sl(0), v1c16)

            # no-edge user fill: xu += nothas (x) mwe   (output half only)
            for ut in range(4):
                ps = psB.tile([P, DM], f32, name="pb", tag="pb", bufs=2)
                nc.tensor.matmul(ps[:], nothas16[0:1, ut * P:(ut + 1) * P],
                                 mwe16[:], start=True, stop=True)
                sl = slice(ut * DM, (ut + 1) * DM)
                nc.vector.tensor_tensor(out=xu16[:, sl], in0=xu16[:, sl],
                                        in1=ps[:], op=OP.add)
            nc.sync.dma_start(
                out=d_ou[0].rearrange("(t p) h -> p t h", p=P),
                in_=xu16[:, 0:4 * DM].rearrange("p (t h) -> p t h", t=4))

            # ---------------- phase 6: day-1 attention ----------------
            h1u16 = persist.tile([P, 8 * DM], f16, name="h1u16")
            for ut in range(8):
                usl = slice(ut * P, (ut + 1) * P)
                ps = psB.tile([P, HD + 1], f32, name="pb", tag="pb", bufs=2)
                for lt in range(8):
                    nc.tensor.matmul(
                        ps[:],
                        PT[:, lt * N_USER + ut * P:lt * N_USER + (ut + 1) * P],
                        whx(lt), start=(lt == 0), stop=False)
                nc.tensor.matmul(ps[:], nothas16[0:1, usl], mw0e16[:],
                                 start=False, stop=True)
                zr = work.tile([P, 1], f32, name="zr", tag="zr", bufs=3)
                nc.vector.reciprocal(zr[:], ps[:, HD:HD + 1])
                if ut % 2 == 0:
                    nc.vector.tensor_scalar(
                        out=h1u16[:, ut * DM:(ut + 1) * DM], in0=ps[:, 0:HD],
                        scalar1=zr[:], scalar2=0.0, op0=OP.mult, op1=OP.max)
                else:
                    nc.scalar.activation(h1u16[:, ut * DM:(ut + 1) * DM],
                                         ps[:, 0:HD], AF.Relu, scale=zr[:])
            nc.sync.dma_start(
                out=d_ou[1].rearrange("(t p) h -> p t h", p=P),
                in_=h1u16[:, 0:4 * DM].rearrange("p (t h) -> p t h", t=4))

            # ---------------- phase 7: days 2..4 ----------------
            scol = [persist.tile([P, 1], f32, name=f"scol0_{i}")
                    for i in range(2)]
            for mt in range(2):
                ps = psC.tile([P, 1], f32, name="ps1", tag="ps1", bufs=2)
                for ut in range(8):
                    nc.tensor.matmul(
                        ps[:], h1u16[:, ut * DM + mt * P:ut * DM + (mt + 1) * P],
                        ones32c[:], start=(ut == 0), stop=(ut == 7))
                nc.vector.tensor_copy(scol[mt][:], ps[:])

            vcol_cur, scol_cur = v1c16, scol
            vs_cur = None
            for day in (2, 3, 4):
                dd = day - 2
                if vs_cur is None:
                    vs2 = [work.tile([P, 2], f16, name=f"vs{day}_{kt}",
                                     tag=f"vs{day}_{kt}") for kt in range(2)]
                    for kt in range(2):
                        nc.vector.tensor_copy(vs2[kt][:, 0:1], vcol_cur[kt][:])
                        nc.vector.tensor_copy(vs2[kt][:, 1:2], scol_cur[kt][:])
                else:
                    vs2 = vs_cur
                ps2 = psB.tile([2, HD], f32, name="pb", tag="pb", bufs=2)
                for kt in range(2):
                    nc.tensor.matmul(ps2[:], vs2[kt][:], W16k(kt),
                                     start=(kt == 0), stop=(kt == 1))
                stk16 = work.tile([2, HD], f16, name=f"stk{day}",
                                  tag=f"stk{day}")
                nc.scalar.copy(stk16[:], ps2[:])
                # r1 row (index 1 + 2*dd), vn row (index 2 + 2*dd)
                nc.scalar.activation(row_sl(1 + 2 * dd), ps2[0:1, :],
                                     AF.Relu)
                psm2 = psB.tile([1, 512], f32, name="psr", tag="pb", bufs=2)
                nc.tensor.matmul(psm2[:, 0:HD], cpair16[:], stk16[:],
                                 start=True, stop=True)
                nc.scalar.activation(row_sl(2 + 2 * dd), psm2[0:1, 0:HD],
                                     AF.Relu)
                if day < 4:
                    # next vs2 = [v | s] built in one [128,2] PSUM tile per
                    # half: col 0 = transpose(vn), col 1 = cnt*r1+(N-cnt)*vn
                    vs_n = [work.tile([P, 2], f16, name=f"vsn{day}_{k}",
                                      tag=f"vsn{day}_{k}") for k in range(2)]
                    for mt in range(2):
                        ps = psC.tile([P, 2], f32, name="ps2c", tag="ps1",
                                      bufs=2)
                        msl = slice(mt * P, (mt + 1) * P)
                        nc.tensor.matmul(ps[:, 0:1],
                                         row_sl(2 + 2 * dd)[:, msl],
                                         one11_16[:], start=True, stop=True)
                        nc.tensor.matmul(ps[:, 1:2],
                                         row_sl(1 + 2 * dd)[:, msl],
                                         ner16[:, dd * 2:dd * 2 + 1],
                                         start=True, stop=False)
                        nc.tensor.matmul(ps[:, 1:2],
                                         row_sl(2 + 2 * dd)[:, msl],
                                         ner16[:, dd * 2 + 1:dd * 2 + 2],
                                         start=False, stop=True)
                        nc.vector.tensor_copy(vs_n[mt][:], ps[:])
                    vcol_cur, scol_cur = None, None
                    vs_cur = vs_n

            nc.sync.dma_start(out=d_rows[:], in_=rows_sb[:])

    return nc


def _host_prep(x_loc, mob_links, text_links, W, a):
    """Index/cast-only preprocessing -> per-core input maps."""
    x_loc = np.ascontiguousarray(x_loc, np.float32)
    W = np.ascontiguousarray(W, np.float32)
    a = np.ascontiguousarray(a, np.float32)
    mob = np.asarray(mob_links)
    text = np.asarray(text_links)

    import ml_dtypes
    shared = {
        "xloc8": x_loc.astype(ml_dtypes.float8_e4m3),
        "xlocT16": np.ascontiguousarray(x_loc.T).astype(np.float16),
        "W16": W.astype(np.float16),
        "WT16": np.ascontiguousarray(W.T).astype(np.float16),
        "acol16": a.reshape(P, 4, order="F").astype(np.float16),
        "cpair16": np.array([[2048.0 / 3072.0], [1.0 / 3072.0]], np.float16),
    }
    # acol layout: [128,4] where col (ai+kt) = a[ai*128... wait:
    # matmul uses acol16[:, ai+kt] with lhsT=WT16k(kt) rows kt*128..].
    # Need col j = a[j*128:(j+1)*128] -> order="F" reshape of [512,1] gives
    # exactly that.

    in_maps = []
    for c in range(NCORES):
        b, r = c // 2, c % 2
        rot = r * 512
        u0 = np.concatenate([mob[b, 0, :, 0], text[b, 0, :, 0]]).astype(np.int64)
        l0 = np.concatenate([mob[b, 0, :, 1], text[b, 0, :, 1]]).astype(np.int64)
        cnt = np.bincount(u0, minlength=N_USER).astype(np.float32)
        A = np.zeros((N_USER, N_LOC), np.float32)
        np.add.at(A, (u0, l0), 1.0)
        Ahat = A / np.maximum(cnt, 1.0)[:, None]
        Mb = np.zeros((N_USER, N_LOC), np.float32)
        Tb = np.zeros((N_USER, N_LOC), np.float32)
        Mb[mob[b, 0, :, 0], mob[b, 0, :, 1]] = 1.0
        Tb[text[b, 0, :, 0], text[b, 0, :, 1]] = 1.0
        M = Mb + Tb
        has0 = (cnt > 0).astype(np.float32)
        n_with = max(float(has0.sum()), 1.0)
        nh_cnt = float(N_USER) - float(has0.sum())

        def rollu(x, axis=0):
            return np.roll(x, -rot, axis=axis)

        ner = np.zeros((1, 6), np.float32)
        for dd in range(3):
            us = np.concatenate([mob[b, dd + 1, :, 0], text[b, dd + 1, :, 0]])
            hasd = np.zeros(N_USER, np.float32)
            hasd[us] = 1.0
            ner[0, 2 * dd] = hasd.sum()
            ner[0, 2 * dd + 1] = N_USER - hasd.sum()
        nothas = rollu(1.0 - has0)[None, :].astype(np.float32)
        hw = (rollu(has0) / n_with).astype(np.float32)
        uw3 = np.stack([hw, np.ones(N_USER, np.float32),
                        np.zeros(N_USER, np.float32)], axis=1)
        import ml_dtypes
        f8 = ml_dtypes.float8_e4m3
        cnt_r = rollu(cnt)
        cinv = (1.0 / np.maximum(cnt_r, 1.0)).astype(np.float32)
        m = dict(shared)
        m.update({
            "AT8": np.ascontiguousarray(rollu(A, 0).T).astype(f8),
            "MT8": np.ascontiguousarray(rollu(M, 0).T).astype(f8),
            "cntc": np.ascontiguousarray(cinv.reshape(8, P).T),
            "cnt2": np.stack([cinv, np.ones(N_USER, np.float32)]).astype(np.float16),
            "nothas16": nothas.astype(np.float16),
            "uw316": np.ascontiguousarray(uw3).astype(np.float16),
            "hcol3": np.array([[nh_cnt / 3072.0], [1.0 / 3072.0],
                               [2.0 / 3072.0]], np.float32),
            "ner16": ner.astype(np.float16),
        })
        in_maps.append(m)
    return in_maps


def kernel(**inputs):
    from concourse.bass_utils import run_bass_kernel_spmd

    if "nc" not in _CACHE:
        _CACHE["nc"] = _build_nc()
    nc = _CACHE["nc"]

    x_loc = np.ascontiguousarray(inputs["x_loc"], np.float32)
    mob = np.asarray(inputs["mob_links"])
    text = np.asarray(inputs["text_links"])
    in_maps = _host_prep(x_loc, mob, text, inputs["W"], inputs["a"])
    res = run_bass_kernel_spmd(nc, in_maps, core_ids=list(range(NCORES)))

    out = np.zeros((B, D, N_USER + 2 * N_LOC, HD), np.float32)
    for c in range(NCORES):
        b, r = c // 2, c % 2
        rot = r * 512
        o_u = np.asarray(res.results[c]["out_u"], np.float32)
        g = slice(rot, rot + 512)
        out[b, 0, g] = o_u[0]
        out[b, 1, g] = o_u[1]
        if r == 0:
            rows = np.asarray(res.results[c]["out_rows"],
                              np.float32).reshape(8, HD)
            v1 = rows[0]
            out[b, 0, N_USER:N_USER + N_LOC] = x_loc
            out[b, 0, N_USER + N_LOC:] = x_loc
            out[b, 1, N_USER:] = v1
            for dd, day in enumerate((2, 3, 4)):
                r1 = rows[1 + 2 * dd]
                vn = rows[2 + 2 * dd]
                us = np.concatenate([mob[b, dd + 1, :, 0],
                                     text[b, dd + 1, :, 0]])
                hasd = np.zeros(N_USER, bool)
                hasd[us] = True
                out[b, day, :N_USER] = np.where(hasd[:, None], r1, vn)
                out[b, day, N_USER:] = vn
    return out
